# revision 1
# baseline (speedup 1.0000x reference)
"""Trainium2 Bass kernel for nn_DistiledMultiheadAttention_76476187673064.

Sliding-window (W=32) single-query attention over ragged sequences with a
learned pre-context buffer, plus input/output projections.

Strategy (8 NeuronCores, data-parallel over flat tokens):
  - Each core owns 512 tokens; kv for a 31-token halo is recomputed locally
    (plus one masked pad column), so no collectives are needed.
  - Host passes transposed weights/activations so every matmul's contraction
    dim lands on SBUF partitions with zero on-device transposes:
      * K/Q projections feature-major (k_T, q_T: [feat, tok])
      * V projection token-major, augmented with a ones column per head
        (so PV emits per-head softmax sums for free)
      * QK logits ctx-major [ctx, tok]; band+segment+buffer masking via a
        host-precomputed additive mask; exp without max-subtraction
        (logits are bounded); PV ctx-major -> o feature-major
      * softmax normalization: reciprocal of the sums row, broadcast across
        partitions with a rank-1 matmul (ones ⊗ r), applied as the PSUM
        eviction multiply
      * biases via rank-1 matmuls accumulated into PSUM (ones ⊗ bias-row)
  - All matmuls run as float32r (full-rate fp32 storage) via AP bitcast.
"""
import math
import sys

sys.path.insert(0, "/opt/trn_rl_repo")

import numpy as np

# ---------------------------------------------------------------- constants
T = 4096
E = 1024
KD = 512          # key dim
H = 16            # heads
W = 32            # window
DK = KD // H      # 32
DV = E // H       # 64
B = 8
MAXL = 768
N_CORES = 8
SHARD = T // N_CORES          # 512 tokens per core
HALO = W - 1                  # 31
NTOK = SHARD + HALO + 1       # 544 token columns incl. halo + 1 pad
TB = 256                      # attention token block
NB = SHARD // TB              # 2 blocks per core
CTXF = 1024                   # logits free: band [128,512] + tail [32,256] + buf [32,256]
NEG = -30000.0

_CACHE = {}


# ------------------------------------------------------------- tile patches
def _apply_tile_patches():
    """This container's walrus only supports ONE sync-wait per instruction;
    redistribute extra Tile-assigned waits onto single-wait InstNoOp carriers."""
    import concourse.mybir as mybir
    import concourse.tile as tile
    from concourse.vector_clock import ScopedClock

    if getattr(tile.TileContext, "_wait_split_patched", False):
        return
    orig_commit = tile.TileContext._commit_and_lower

    def commit_split(self, inst, original_block, old_bb_map, bb_to_exit_bb):
        si = getattr(inst, "sync_info", None)
        if si is not None and si.on_wait and len(si.on_wait) > 1:
            engine = inst.engine
            if engine is not None and engine != mybir.EngineType.Unassigned:
                waits = list(si.on_wait)
                si.on_wait = waits[-1:]
                for w in waits[:-1]:
                    noop = mybir.InstNoOp(
                        name=self.nc.get_next_instruction_name(),
                        sync_info=mybir.SyncInfo(on_wait=[w], on_update=[]),
                        bass_nofuse=True,
                        engine=engine,
                        text_hint="wait_split",
                    )
                    orig_commit(self, noop, original_block, old_bb_map, bb_to_exit_bb)
        return orig_commit(self, inst, original_block, old_bb_map, bb_to_exit_bb)

    def drain_and_barrier(self, tick_clock, wait_clock):
        drain_inst = self.nc.sync.drain()
        wait_clock.add_sem_waits(
            drain_inst.ins, ScopedClock({None: tick_clock.global_clock})
        )
        si = drain_inst.ins.sync_info
        if si is not None and si.on_wait and len(si.on_wait) > 1:
            waits = list(si.on_wait)
            si.on_wait = waits[:1]
            for w in waits[1:]:
                nop = self.nc.sync.nop(nofuse=True)
                nsi = nop.ins.sync_info
                if nsi is None:
                    nop.ins.sync_info = mybir.SyncInfo(on_wait=[w], on_update=[])
                else:
                    nsi.on_wait = list(nsi.on_wait or []) + [w]
        self.nc.all_engine_barrier()
        assert self.sems is not None
        popped = self.nc._tile_sem_poison_stack.pop()
        assert popped is self._sem_poison
        self.nc.clear_and_free_semaphores(list(self.sems.allocated().values()))
        self.nc.all_engine_barrier()

    tile.TileContext._commit_and_lower = commit_split
    tile.TileContext._drain_and_barrier = drain_and_barrier
    tile.TileContext._wait_split_patched = True


def _act_recip(nc, out, in_):
    """ScalarE LUT reciprocal (bass gates ActivationFunctionType.Reciprocal
    behind a ValueError for accuracy; softmax denominators tolerate it —
    verified end-to-end against the reference)."""
    import concourse.mybir as mybir

    eng = nc.scalar
    inputs = [eng.lower_ap(in_)]
    for arg in (0.0, 1.0, 0.0):  # bias, scale, alpha
        inputs.append(mybir.ImmediateValue(dtype=mybir.dt.float32, value=arg))
    return eng.add_instruction(
        mybir.InstActivation(
            name=nc.get_next_instruction_name(),
            func=mybir.ActivationFunctionType.Reciprocal,
            ins=inputs,
            outs=[eng.lower_ap(out)],
        )
    )


# ------------------------------------------------------------- device build
def _build_nc(with_bias=True):
    import concourse.bass as bass
    import concourse.mybir as mybir
    import concourse.tile as tile

    import bass_rust

    _apply_tile_patches()
    f32 = mybir.dt.float32
    f32r = mybir.dt.float32r

    nc = bass.Bass()
    d_xT = nc.dram_tensor("xT", [E, NTOK], f32r, kind="ExternalInput")
    d_wk = nc.dram_tensor("wk", [E, KD], f32r, kind="ExternalInput")
    d_wv = nc.dram_tensor("wv", [E, H * 65], f32r, kind="ExternalInput")
    d_wq = nc.dram_tensor("wq", [E, KD], f32r, kind="ExternalInput")
    d_wp = nc.dram_tensor("wp", [E, E], f32r, kind="ExternalInput")
    d_bk = nc.dram_tensor("bk", [128, 4], f32, kind="ExternalInput")
    d_bq = nc.dram_tensor("bq", [128, 4], f32, kind="ExternalInput")
    d_bv = nc.dram_tensor("bv", [1, H * 65], f32r, kind="ExternalInput")
    d_bp = nc.dram_tensor("bp", [1, E], f32r, kind="ExternalInput")
    d_ones = nc.dram_tensor("ones", [1, 128], f32r, kind="ExternalInput")
    d_kbufT = nc.dram_tensor("kbufT", [KD, 32], f32r, kind="ExternalInput")
    d_vbuf = nc.dram_tensor("vbuf", [32, H * 65], f32r, kind="ExternalInput")
    d_mask = nc.dram_tensor("mask", [NB, 128, CTXF], f32, kind="ExternalInput")
    d_y = nc.dram_tensor("yout", [SHARD, E], f32, kind="ExternalOutput")

    with tile.TileContext(nc) as tc, nc.allow_low_precision(
        reason="f32r matmul operands; fp32 PSUM accumulation throughout"
    ):
        with (
            tc.tile_pool(name="const", bufs=1) as const_pool,
            tc.tile_pool(name="x", bufs=1) as x_pool,
            tc.tile_pool(name="kqv", bufs=1) as kqv_pool,
            tc.tile_pool(name="wsmall", bufs=3) as ws_pool,
            tc.tile_pool(name="wbig", bufs=2) as wb_pool,
            tc.tile_pool(name="exp", bufs=10) as exp_pool,
            tc.tile_pool(name="rrow", bufs=4) as r_pool,
            tc.tile_pool(name="out", bufs=3) as out_pool,
        ):
            # ---- constants
            ones = const_pool.tile([1, 128], f32r)
            nc.sync.dma_start(ones[:], d_ones[:])
            bk_sb = const_pool.tile([128, 4], f32)
            nc.sync.dma_start(bk_sb[:], d_bk[:])
            bq_sb = const_pool.tile([128, 4], f32)
            nc.sync.dma_start(bq_sb[:], d_bq[:])
            bv_sb = const_pool.tile([1, H * 65], f32r)
            nc.sync.dma_start(bv_sb[:], d_bv[:])
            bp_sb = const_pool.tile([1, E], f32r)
            nc.sync.dma_start(bp_sb[:], d_bp[:])
            kbuf_sb = const_pool.tile([128, 4, 32], f32r)
            nc.sync.dma_start(
                kbuf_sb[:], d_kbufT[:].rearrange("(m p) c -> p m c", p=128)
            )
            vbuf_sb = const_pool.tile([32, H * 65], f32r)
            nc.sync.dma_start(vbuf_sb[:], d_vbuf[:])
            mask_sb = const_pool.tile([128, NB, CTXF], f32)
            nc.sync.dma_start(
                mask_sb[:], d_mask[:].rearrange("b p f -> p b f")
            )

            # ---- x (feature-major, all tokens incl. halo+pad)
            xT = x_pool.tile([128, 8, NTOK], f32r)
            nc.sync.dma_start(xT[:], d_xT[:].rearrange("(c p) t -> p c t", p=128))

            # ---- persistent activations
            kT = kqv_pool.tile([128, 4, NTOK], f32r)   # K feature-major
            qT = kqv_pool.tile([128, 4, SHARD], f32r)  # Q feature-major (scaled)
            vA = kqv_pool.tile([128, 5, H * 65], f32r)  # V token-major + ones col
            oT = kqv_pool.tile([128, 8, SHARD], f32r)  # attention out feature-major

            with tc.tile_pool(name="pp1", bufs=2, space="PSUM") as pp1:
                # K projection: kT[:, m, :] = wk[:, mchunk].T @ xT (+ bk)
                for m in range(4):
                    wkt = ws_pool.tile([128, 8, 128], f32r, tag="wkq")
                    nc.sync.dma_start(
                        wkt[:],
                        d_wk[:, m * 128:(m + 1) * 128].rearrange(
                            "(c p) f -> p c f", p=128
                        ),
                    )
                    pa = pp1.tile([128, 512], f32, tag="pa")
                    pb = pp1.tile([128, 32], f32, tag="pb")
                    for e in range(8):
                        nc.tensor.matmul(
                            pa[:], wkt[:, e, :], xT[:, e, 0:512],
                            start=(e == 0), stop=(e == 7),
                        )
                        nc.tensor.matmul(
                            pb[:], wkt[:, e, :], xT[:, e, 512:NTOK],
                            start=(e == 0), stop=(e == 7),
                        )
                    if with_bias:
                        nc.vector.tensor_scalar_add(kT[:, m, 0:512], pa[:], bk_sb[:, m:m + 1])
                        nc.vector.tensor_scalar_add(kT[:, m, 512:NTOK], pb[:], bk_sb[:, m:m + 1])
                    else:
                        nc.vector.tensor_copy(kT[:, m, 0:512], pa[:])
                        nc.vector.tensor_copy(kT[:, m, 512:NTOK], pb[:])

                # Q projection (tokens only, no halo): qT = wq.T @ xT[:, 31:543] (+ bq)
                for m in range(4):
                    wqt = ws_pool.tile([128, 8, 128], f32r, tag="wkq")
                    nc.sync.dma_start(
                        wqt[:],
                        d_wq[:, m * 128:(m + 1) * 128].rearrange(
                            "(c p) f -> p c f", p=128
                        ),
                    )
                    pa = pp1.tile([128, 512], f32, tag="pa")
                    for e in range(8):
                        nc.tensor.matmul(
                            pa[:], wqt[:, e, :], xT[:, e, HALO:HALO + SHARD],
                            start=(e == 0), stop=(e == 7),
                        )
                    if with_bias:
                        nc.vector.tensor_scalar_add(qT[:, m, :], pa[:], bq_sb[:, m:m + 1])
                    else:
                        nc.vector.tensor_copy(qT[:, m, :], pa[:])

                # V projection token-major (wv pre-augmented with ones cols):
                # vA[tok, h*65:h*65+65] = [x @ Wv_h.T + bv_h | 1]
                tok_sizes = [128, 128, 128, 128, 32]
                for f in range(4):
                    wvt = wb_pool.tile([128, 8, 260], f32r, tag="wv")
                    nc.sync.dma_start(
                        wvt[:],
                        d_wv[:, f * 260:(f + 1) * 260].rearrange(
                            "(c p) f2 -> p c f2", p=128
                        ),
                    )
                    for i in range(5):
                        mt = tok_sizes[i]
                        pa = pp1.tile([128, 260], f32, tag="pv")
                        for e in range(8):
                            nc.tensor.matmul(
                                pa[0:mt, :],
                                xT[:, e, i * 128:i * 128 + mt],
                                wvt[:, e, :],
                                start=(e == 0), stop=(e == 7 and not with_bias),
                            )
                        if with_bias:
                            nc.tensor.matmul(
                                pa[0:mt, :], ones[0:1, 0:mt],
                                bv_sb[0:1, f * 260:(f + 1) * 260],
                                start=False, stop=True,
                            )
                        nc.vector.tensor_copy(
                            vA[0:mt, i, f * 260:(f + 1) * 260], pa[0:mt, :]
                        )
                        if not with_bias:
                            # ones columns via strided add (psum zeros there)
                            ov_view = vA[0:mt, i, f * 260:(f + 1) * 260].rearrange(
                                "p (h c) -> p h c", c=65
                            )[:, :, 64:65]
                            nc.vector.tensor_scalar_add(ov_view, ov_view, 1.0)

            # ---- attention
            with (
                tc.tile_pool(name="plg", bufs=2, space="PSUM") as plg,
                tc.tile_pool(name="pov", bufs=2, space="PSUM") as pov,
                tc.tile_pool(name="prb", bufs=2, space="PSUM") as prb,
            ):
                GRP = 8
                prev_last_recip = None
                for b in range(NB):
                    base = b * TB
                    for g in range(H // GRP):
                        exs = []
                        first_exp = None
                        # phase A: QK + mask + exp for the whole group
                        # (one Exp table residency; dense PE matmul burst)
                        for hh in range(GRP):
                            h = g * GRP + hh
                            ro = (h % 4) * 32
                            ht = h // 4
                            lg = plg.tile([128, CTXF], f32)
                            qh = qT[ro:ro + 32, ht, base:base + TB]
                            nc.tensor.matmul(
                                lg[:, 0:256], kT[ro:ro + 32, ht, base:base + 128],
                                qh, start=True, stop=True, tile_position=(ro, 0),
                            )
                            nc.tensor.matmul(
                                lg[:, 256:512], kT[ro:ro + 32, ht, base + 128:base + 256],
                                qh, start=True, stop=True, tile_position=(ro, 0),
                            )
                            nc.tensor.matmul(
                                lg[0:32, 512:768], kT[ro:ro + 32, ht, base + 256:base + 288],
                                qh, start=True, stop=True, tile_position=(ro, 0),
                            )
                            nc.tensor.matmul(
                                lg[0:32, 768:1024], kbuf_sb[ro:ro + 32, ht, :],
                                qh, start=True, stop=True, tile_position=(ro, 0),
                            )
                            nc.vector.tensor_tensor(
                                lg[:, 0:512], lg[:, 0:512], mask_sb[:, b, 0:512],
                                mybir.AluOpType.add,
                            )
                            nc.vector.tensor_tensor(
                                lg[0:32, 512:1024], lg[0:32, 512:1024],
                                mask_sb[0:32, b, 512:1024], mybir.AluOpType.add,
                            )
                            ex = exp_pool.tile([128, CTXF], f32r)
                            e1 = nc.scalar.activation(
                                ex[:, 0:512], lg[:, 0:512],
                                mybir.ActivationFunctionType.Exp,
                            )
                            if first_exp is None:
                                first_exp = e1
                                if prev_last_recip is not None:
                                    # keep ACT's Exp/Reciprocal LUT loads
                                    # batched per phase (ordering-only dep)
                                    bass_rust.add_dep_helper(
                                        e1.ins, prev_last_recip.ins,
                                        sync=False,
                                        reason="ACT table residency batching",
                                    )
                            nc.scalar.activation(
                                ex[0:32, 512:1024], lg[0:32, 512:1024],
                                mybir.ActivationFunctionType.Exp,
                            )
                            exs.append(ex)
                        # phase B: PV + normalization for the group
                        # (one Reciprocal table residency)
                        for hh in range(GRP):
                            h = g * GRP + hh
                            ex = exs[hh]
                            ov = pov.tile([128, TB], f32)
                            hc = h * 65
                            nc.tensor.matmul(
                                ov[0:65, :], vA[:, 2 * b, hc:hc + 65],
                                ex[:, 0:256], start=True, stop=False,
                            )
                            nc.tensor.matmul(
                                ov[0:65, :], vA[:, 2 * b + 1, hc:hc + 65],
                                ex[:, 256:512], start=False, stop=False,
                            )
                            nc.tensor.matmul(
                                ov[0:65, :], vA[0:32, 2 * b + 2, hc:hc + 65],
                                ex[0:32, 512:768], start=False, stop=False,
                            )
                            nc.tensor.matmul(
                                ov[0:65, :], vbuf_sb[:, hc:hc + 65],
                                ex[0:32, 768:1024], start=False, stop=True,
                            )
                            sr = r_pool.tile([1, TB], f32r)
                            prev_last_recip = _act_recip(nc, sr[:], ov[64:65, :])
                            rb = prb.tile([64, TB], f32)
                            nc.tensor.matmul(
                                rb[:], ones[0:1, 0:64], sr[:],
                                start=True, stop=True,
                            )
                            od = oT[(h % 2) * 64:(h % 2) * 64 + 64, h // 2,
                                    base:base + TB]
                            nc.vector.tensor_copy(od, ov[0:64, :])
                            nc.vector.tensor_tensor(
                                od, od, rb[:], mybir.AluOpType.mult,
                            )

            # ---- output projection: y[tok, :] = oT.T @ wp (+ bp)
            with tc.tile_pool(name="pp3", bufs=2, space="PSUM") as pp3:
                for f in range(2):
                    wpt = wb_pool.tile([128, 8, 512], f32r, tag="wbig")
                    nc.sync.dma_start(
                        wpt[:],
                        d_wp[:, f * 512:(f + 1) * 512].rearrange(
                            "(c p) f2 -> p c f2", p=128
                        ),
                    )
                    for m in range(4):
                        pa = pp3.tile([128, 512], f32)
                        for c in range(8):
                            nc.tensor.matmul(
                                pa[:], oT[:, c, m * 128:(m + 1) * 128],
                                wpt[:, c, :], start=(c == 0),
                                stop=(c == 7 and not with_bias),
                            )
                        if with_bias:
                            nc.tensor.matmul(
                                pa[:], ones[0:1, 0:128],
                                bp_sb[0:1, f * 512:(f + 1) * 512],
                                start=False, stop=True,
                            )
                        ot = out_pool.tile([128, 512], f32)
                        nc.vector.tensor_copy(ot[:], pa[:])
                        nc.sync.dma_start(
                            d_y[m * 128:(m + 1) * 128, f * 512:(f + 1) * 512], ot[:]
                        )
    return nc


def _get_runner(with_bias=True):
    key = ("runner", with_bias)
    if key in _CACHE:
        return _CACHE[key]
    import jax
    import concourse.mybir as mybir
    from concourse import bass2jax
    from jax.sharding import Mesh, PartitionSpec
    from jax.experimental.shard_map import shard_map

    nc = _build_nc(with_bias)
    bass2jax.install_neuronx_cc_hook()
    partition_name = nc.partition_id_tensor.name if nc.partition_id_tensor else None
    in_names, out_names, out_avals, out_shapes = [], [], [], []
    for alloc in nc.m.functions[0].allocations:
        if not isinstance(alloc, mybir.MemoryLocationSet):
            continue
        name = alloc.memorylocations[0].name
        if alloc.kind == "ExternalInput":
            if name != partition_name:
                in_names.append(name)
        elif alloc.kind == "ExternalOutput":
            shape = tuple(alloc.tensor_shape)
            dtype = mybir.dt.np(alloc.dtype)
            out_names.append(name)
            out_avals.append(jax.core.ShapedArray(shape, dtype))
            out_shapes.append((shape, dtype))
    n_params = len(in_names)
    n_outs = len(out_avals)
    all_in_names = in_names + out_names + ([partition_name] if partition_name else [])
    donate = tuple(range(n_params, n_params + n_outs))

    def _body(*args):
        operands = list(args)
        if partition_name is not None:
            operands.append(bass2jax.partition_id_tensor())
        outs = bass2jax._bass_exec_p.bind(
            *operands,
            out_avals=tuple(out_avals),
            in_names=tuple(all_in_names),
            out_names=tuple(out_names),
            lowering_input_output_aliases=(),
            sim_require_finite=True,
            sim_require_nnan=True,
            nc=nc,
        )
        return tuple(outs)

    devices = jax.devices()[:N_CORES]
    mesh = Mesh(np.asarray(devices), ("core",))
    sharded = jax.jit(
        shard_map(
            _body, mesh=mesh,
            in_specs=(PartitionSpec("core"),) * (n_params + n_outs),
            out_specs=(PartitionSpec("core"),) * n_outs,
            check_rep=False,
        ),
        donate_argnums=donate,
        keep_unused=True,
    )

    def run(in_maps):
        per_core = [[np.asarray(m[name]) for name in in_names] for m in in_maps]
        concat_in = [
            np.concatenate([per_core[c][i] for c in range(N_CORES)], axis=0)
            for i in range(n_params)
        ]
        concat_zeros = [
            np.zeros((N_CORES * s[0], *s[1:]), d) for (s, d) in out_shapes
        ]
        out_arrs = sharded(*concat_in, *concat_zeros)
        return [
            {
                name: np.asarray(out_arrs[i]).reshape(N_CORES, *out_shapes[i][0])[c]
                for i, name in enumerate(out_names)
            }
            for c in range(N_CORES)
        ]

    _CACHE[key] = run
    return run


# ------------------------------------------------------------------- host
def _prep_inputs(x, Wkv, bkv, Wq, bq, Wp, bp, buffer, sample_lengths):
    x = np.asarray(x, np.float32)
    Wkv = np.asarray(Wkv, np.float32)
    bkv = np.asarray(bkv, np.float32)
    Wq = np.asarray(Wq, np.float32)
    bq = np.asarray(bq, np.float32)
    Wp = np.asarray(Wp, np.float32)
    bp = np.asarray(bp, np.float32)
    buffer = np.asarray(buffer, np.float32)
    lengths = np.asarray(sample_lengths).astype(np.int64)

    scale = 1.0 / math.sqrt(DK)
    starts = np.concatenate([[0], np.cumsum(lengths)[:-1]]).astype(np.int64)
    t = np.arange(T)
    seg = np.searchsorted(starts, t, side="right") - 1
    j = t - starts[seg]

    wk = np.ascontiguousarray(Wkv[:KD, :].T)
    wv_aug = np.zeros((E, H, 65), np.float32)
    wv_aug[:, :, :64] = Wkv[KD:, :].T.reshape(E, H, DV)
    wv = np.ascontiguousarray(wv_aug.reshape(E, H * 65))
    wq = np.ascontiguousarray(Wq.T * scale)
    wp = np.ascontiguousarray(Wp.T)
    bk2 = np.ascontiguousarray(bkv[:KD].reshape(4, 128).T)
    bq2 = np.ascontiguousarray((bq * scale).reshape(4, 128).T)
    bv_aug = np.zeros((H, 65), np.float32)
    bv_aug[:, :64] = bkv[KD:].reshape(H, DV)
    bv_aug[:, 64] = 1.0
    bv_row = np.ascontiguousarray(bv_aug.reshape(1, H * 65))
    bp_row = np.ascontiguousarray(bp[None, :])
    ones_row = np.ones((1, 128), np.float32)

    kbufT = np.zeros((KD, 32), np.float32)
    kbufT[:, :HALO] = buffer[:, :KD].T
    vbuf = np.zeros((32, H * 65), np.float32)
    vb = vbuf.reshape(32, H, 65)
    vb[:HALO, :, :64] = buffer[:, KD:].reshape(HALO, H, DV)
    vb[:HALO, :, 64] = 1.0

    xTp = np.zeros((E, T + HALO + 33), np.float32)
    xTp[:, HALO:HALO + T] = x.T

    in_maps = []
    for c in range(N_CORES):
        t0 = c * SHARD
        xT_c = np.ascontiguousarray(xTp[:, t0:t0 + NTOK])
        mask = np.full((NB, 128, CTXF), NEG, np.float32)
        for bblk in range(NB):
            i = np.arange(TB)
            tt = t0 + bblk * TB + i
            st = starts[seg[tt]]
            jj = j[tt]
            for r in range(2):
                p = np.arange(128)[:, None]
                g = t0 - HALO + bblk * TB + r * 128 + p
                valid = (
                    (g >= tt[None, :] - HALO) & (g <= tt[None, :])
                    & (g >= st[None, :]) & (g >= 0) & (g < T)
                )
                mask[bblk, :, r * 256:(r + 1) * 256] = np.where(valid, 0.0, NEG)
            p = np.arange(32)[:, None]
            g = t0 - HALO + bblk * TB + 256 + p
            valid = (
                (g >= tt[None, :] - HALO) & (g <= tt[None, :])
                & (g >= st[None, :]) & (g >= 0) & (g < T)
            )
            mask[bblk, 0:32, 512:768] = np.where(valid, 0.0, NEG)
            pb = np.arange(32)[:, None]
            validb = (pb >= jj[None, :]) & (pb <= HALO - 1)
            mask[bblk, 0:32, 768:1024] = np.where(validb, 0.0, NEG)
        in_maps.append({
            "xT": xT_c, "wk": wk, "wv": wv, "wq": wq, "wp": wp,
            "bk": bk2, "bq": bq2, "bv": bv_row, "bp": bp_row,
            "ones": ones_row, "kbufT": kbufT, "vbuf": vbuf,
            "mask": np.ascontiguousarray(mask),
        })
    return in_maps, seg, j


def kernel(x, Wkv, bkv, Wq, bq, Wp, bp, buffer, sample_lengths):
    in_maps, seg, j = _prep_inputs(
        x, Wkv, bkv, Wq, bq, Wp, bp, buffer, sample_lengths
    )
    with_bias = bool(
        np.any(np.asarray(bkv)) or np.any(np.asarray(bq)) or np.any(np.asarray(bp))
    )
    run = _get_runner(with_bias)
    results = run(in_maps)
    out_full = np.concatenate([results[c]["yout"] for c in range(N_CORES)], axis=0)
    y = np.zeros((B, MAXL, E), np.float32)
    ok = j < MAXL
    y[seg[ok], j[ok]] = out_full[ok]
    return y



# revision 28
# speedup vs baseline: 1.4700x; 1.4700x over previous
"""Trainium2 Bass kernel for nn_DistiledMultiheadAttention_76476187673064.

Sliding-window (W=32) single-query attention over ragged sequences with a
learned pre-context buffer, plus input/output projections.

Strategy (8 NeuronCores, data-parallel over flat tokens):
  - Each core owns 512 tokens plus a 32-token front halo (recomputed
    locally from the neighbour's x slice, so no collectives).
  - bf16 everywhere (verified 3.5e-3 rel err vs the 2e-2 gate): matmuls
    run 1 cycle/row at ANY moving size, DMA halves.
  - Attention in 64-token blocks: the whole window band for a block is
    96 data slots (tokens b*64-32 .. b*64+63) + 32 buffer slots = 128
    partitions exactly. Logits for 8 heads x 64 tokens live in one
    [128, 512] PSUM bank:
      * band/segment/buffer mask added via identity-matmul on the PE
        (start=True writes the mask, QK matmuls accumulate onto it)
      * one Exp per tile (Scalar keeps its table resident all kernel)
      * softmax sums via a ones-column matmul, reciprocal on the Vector
        engine (reciprocal_approx_fast), normalization applied to the
        probabilities via a rank-1 broadcast matmul + one multiply
      * PV: one matmul per (head, block) with full-128 contraction
        against per-block V tiles (built by SBUF->SBUF DMA, buffer rows
        appended); outputs packed per head-pair into [128, 512] banks
        that evict straight into the feature-major oT layout.
  - All weights/x host-packed into the exact SBUF layouts so every DMA
    is a contiguous per-partition block.
"""
import math
import sys

sys.path.insert(0, "/opt/trn_rl_repo")

import numpy as np

# ---------------------------------------------------------------- constants
T = 4096
E = 1024
KD = 512          # key dim
H = 16            # heads
W = 32            # window
DK = KD // H      # 32
DV = E // H       # 64
B = 8
MAXL = 768
N_CORES = 8
SHARD = T // N_CORES          # 512 tokens per core
HALO = 32                     # front halo (31 window + 1 alignment pad)
NTOK = SHARD + HALO           # 544 token slots incl. halo
BLK = 64                      # attention block tokens
NBLK = SHARD // BLK           # 8
NEG = -30000.0

_CACHE = {}


# ------------------------------------------------------------- tile patches
def _apply_tile_patches():
    """This container's walrus only supports ONE sync-wait per instruction;
    redistribute extra Tile-assigned waits onto single-wait InstNoOp carriers."""
    import concourse.mybir as mybir
    import concourse.tile as tile
    from concourse.vector_clock import ScopedClock

    if getattr(tile.TileContext, "_wait_split_patched", False):
        return
    orig_commit = tile.TileContext._commit_and_lower

    def commit_split(self, inst, original_block, old_bb_map, bb_to_exit_bb):
        si = getattr(inst, "sync_info", None)
        if si is not None and si.on_wait and len(si.on_wait) > 1:
            engine = inst.engine
            if engine is not None and engine != mybir.EngineType.Unassigned:
                waits = list(si.on_wait)
                si.on_wait = waits[-1:]
                for w in waits[:-1]:
                    noop = mybir.InstNoOp(
                        name=self.nc.get_next_instruction_name(),
                        sync_info=mybir.SyncInfo(on_wait=[w], on_update=[]),
                        bass_nofuse=True,
                        engine=engine,
                        text_hint="wait_split",
                    )
                    orig_commit(self, noop, original_block, old_bb_map, bb_to_exit_bb)
        return orig_commit(self, inst, original_block, old_bb_map, bb_to_exit_bb)

    def drain_and_barrier(self, tick_clock, wait_clock):
        drain_inst = self.nc.sync.drain()
        wait_clock.add_sem_waits(
            drain_inst.ins, ScopedClock({None: tick_clock.global_clock})
        )
        si = drain_inst.ins.sync_info
        if si is not None and si.on_wait and len(si.on_wait) > 1:
            waits = list(si.on_wait)
            si.on_wait = waits[:1]
            for w in waits[1:]:
                nop = self.nc.sync.nop(nofuse=True)
                nsi = nop.ins.sync_info
                if nsi is None:
                    nop.ins.sync_info = mybir.SyncInfo(on_wait=[w], on_update=[])
                else:
                    nsi.on_wait = list(nsi.on_wait or []) + [w]
        self.nc.all_engine_barrier()
        assert self.sems is not None
        popped = self.nc._tile_sem_poison_stack.pop()
        assert popped is self._sem_poison
        self.nc.clear_and_free_semaphores(list(self.sems.allocated().values()))
        self.nc.all_engine_barrier()

    tile.TileContext._commit_and_lower = commit_split
    tile.TileContext._drain_and_barrier = drain_and_barrier
    tile.TileContext._wait_split_patched = True


# ------------------------------------------------------------- device build
def _build_nc(with_bias=True):
    import concourse.bass as bass
    import concourse.mybir as mybir
    import concourse.tile as tile
    from concourse.dve_ops import (
        RECIP_APPROX_FAST_CONSTS,
        RECIPROCAL_APPROX_FAST,
    )

    _apply_tile_patches()
    f32 = mybir.dt.float32
    f32r = mybir.dt.float32r
    bf16 = mybir.dt.bfloat16

    nc = bass.Bass()
    # device-layout dram tensors (host packs exactly these shapes)
    d_xT = nc.dram_tensor("xT", [128, 8 * NTOK], bf16, kind="ExternalInput")
    d_wk = nc.dram_tensor("wk", [128, 4 * 8 * 128], bf16, kind="ExternalInput")
    d_wq = nc.dram_tensor("wq", [128, 4 * 8 * 128], bf16, kind="ExternalInput")
    d_wv = nc.dram_tensor("wv", [128, 8 * 1024], bf16, kind="ExternalInput")
    d_wp = nc.dram_tensor("wp", [128, 8 * 1024], bf16, kind="ExternalInput")
    d_mask = nc.dram_tensor("mask", [128, (NBLK // 2) * 512], bf16, kind="ExternalInput")
    d_kbuf = nc.dram_tensor("kbuf", [128, 4 * 32], bf16, kind="ExternalInput")
    d_vbuf = nc.dram_tensor("vbuf", [32, 1024], bf16, kind="ExternalInput")
    d_ident = nc.dram_tensor("ident", [128, 128], bf16, kind="ExternalInput")
    d_onec = nc.dram_tensor("onec", [128, 1], bf16, kind="ExternalInput")
    d_oner = nc.dram_tensor("oner", [1, 128], f32r, kind="ExternalInput")
    d_oner16 = nc.dram_tensor("oner16", [1, 128], bf16, kind="ExternalInput")
    d_bk = nc.dram_tensor("bk", [128, 4], f32, kind="ExternalInput")
    d_bq = nc.dram_tensor("bq", [128, 4], f32, kind="ExternalInput")
    d_bv = nc.dram_tensor("bv", [1, 1024], f32r, kind="ExternalInput")
    d_bp = nc.dram_tensor("bp", [1, E], f32r, kind="ExternalInput")
    d_y = nc.dram_tensor("yout", [SHARD, E], f32, kind="ExternalOutput")

    with tile.TileContext(nc) as tc, nc.allow_low_precision(
        reason="bf16 operands; fp32 PSUM accumulation throughout"
    ):
        with (
            tc.tile_pool(name="const", bufs=1) as const_pool,
            tc.tile_pool(name="x", bufs=1) as x_pool,
            tc.tile_pool(name="wkq", bufs=1) as wkq_pool,
            tc.tile_pool(name="wvp", bufs=1) as wvp_pool,
            tc.tile_pool(name="act", bufs=1) as act_pool,
            tc.tile_pool(name="rr", bufs=3) as rr_pool,
            tc.tile_pool(name="out", bufs=3) as out_pool,
            tc.tile_pool(name="scr", bufs=1, space="DRAM") as scr_pool,
        ):
            # ---- constants / small tiles
            ident = const_pool.tile([128, 128], bf16)
            nc.sync.dma_start(ident[:], d_ident[:])
            onec = const_pool.tile([128, 1], bf16)
            nc.sync.dma_start(onec[:], d_onec[:])
            oner = const_pool.tile([1, 128], f32r)
            nc.sync.dma_start(oner[:], d_oner[:])
            oner16 = const_pool.tile([1, 128], bf16)
            nc.sync.dma_start(oner16[:], d_oner16[:])
            mask_sb = const_pool.tile([128, NBLK // 2, 512], bf16)
            nc.sync.dma_start(
                mask_sb[:], d_mask[:].rearrange("p (b f) -> p b f", b=NBLK // 2)
            )
            if with_bias:
                bk_sb = const_pool.tile([128, 4], f32)
                nc.sync.dma_start(bk_sb[:], d_bk[:])
                bq_sb = const_pool.tile([128, 4], f32)
                nc.sync.dma_start(bq_sb[:], d_bq[:])
                bv_sb = const_pool.tile([1, 1024], f32r)
                nc.sync.dma_start(bv_sb[:], d_bv[:])
                bp_sb = const_pool.tile([1, E], f32r)
                nc.sync.dma_start(bp_sb[:], d_bp[:])

            # ---- x (feature-major, halo + tokens)
            xT = x_pool.tile([128, 8, NTOK], bf16)
            nc.sync.dma_start(xT[:], d_xT[:].rearrange("p (c t) -> p c t", c=8))

            # ---- persistent activations
            kT = act_pool.tile([128, 4, NTOK], bf16)    # K feature-major
            qT = act_pool.tile([128, 4, SHARD], bf16)   # Q feature-major (scaled)
            vA = act_pool.tile([128, 5, 1024], bf16)    # V token-major chunks
            kTb = act_pool.tile([128, NBLK, 4, 128], bf16)  # per-block K slots
            vblk = act_pool.tile([128, NBLK, 1024], bf16)   # per-block V slots
            exn = act_pool.tile([128, 2 * NBLK, 512], bf16)  # probabilities
            oT = act_pool.tile([128, 8, SHARD], bf16)   # attn out feature-major

            # ---- K projection: kT[:, m, :] = wk_m.T @ xT  (feature-major)
            wkt = wkq_pool.tile([128, 4, 8, 128], bf16, tag="wk")
            nc.sync.dma_start(
                wkt[:], d_wk[:].rearrange("p (m e f) -> p m e f", m=4, e=8)
            )
            wqt = wkq_pool.tile([128, 4, 8, 128], bf16, tag="wq")
            nc.sync.dma_start(
                wqt[:], d_wq[:].rearrange("p (m e f) -> p m e f", m=4, e=8)
            )
            with tc.tile_pool(name="ppk", bufs=2, space="PSUM") as ppk:
                for m in range(4):
                    pa = ppk.tile([128, 512], f32, tag="pa")
                    pb = ppk.tile([128, 32], f32, tag="pb")
                    for e in range(8):
                        nc.tensor.matmul(
                            pa[:], wkt[:, m, e, :], xT[:, e, 0:512],
                            start=(e == 0), stop=(e == 7),
                        )
                        nc.tensor.matmul(
                            pb[:], wkt[:, m, e, :], xT[:, e, 512:NTOK],
                            start=(e == 0), stop=(e == 7),
                        )
                    if with_bias:
                        nc.scalar.activation(
                            kT[:, m, 0:512], pa[:],
                            mybir.ActivationFunctionType.Identity,
                            bias=bk_sb[:, m:m + 1],
                        )
                        nc.scalar.activation(
                            kT[:, m, 512:NTOK], pb[:],
                            mybir.ActivationFunctionType.Identity,
                            bias=bk_sb[:, m:m + 1],
                        )
                    else:
                        nc.scalar.copy(kT[:, m, 0:512], pa[:])
                        nc.scalar.copy(kT[:, m, 512:NTOK], pb[:])

                # Q projection (tokens only): qT = wq_m.T @ xT[:, HALO:]
                for m in range(4):
                    pa = ppk.tile([128, 512], f32, tag="pa")
                    for e in range(8):
                        nc.tensor.matmul(
                            pa[:], wqt[:, m, e, :], xT[:, e, HALO:HALO + SHARD],
                            start=(e == 0), stop=(e == 7),
                        )
                    if with_bias:
                        nc.scalar.activation(
                            qT[:, m, :], pa[:],
                            mybir.ActivationFunctionType.Identity,
                            bias=bq_sb[:, m:m + 1],
                        )
                    else:
                        nc.scalar.copy(qT[:, m, :], pa[:])

            # per-block K slot tiles: data cols 0:96 from kT (via DRAM
            # round-trip; SBUF->SBUF DMA faults on this runtime), buffer 96:128
            scrk = scr_pool.tile([128, 4, NTOK], bf16)
            nc.sync.dma_start(scrk[:], kT[:])
            for b in range(NBLK):
                nc.sync.dma_start(
                    kTb[:, b, :, 0:96], scrk[:, :, b * BLK:b * BLK + 96]
                )
                nc.sync.dma_start(
                    kTb[:, b, :, 96:128],
                    d_kbuf[:].rearrange("p (m c) -> p m c", m=4),
                )

            # ---- V projection token-major: vA[tok, :] = x @ Wv.T
            with tc.tile_pool(name="ppv", bufs=2, space="PSUM") as ppv:
                wvt = wvp_pool.tile([128, 8, 1024], bf16, tag="wv")
                nc.sync.dma_start(
                    wvt[:], d_wv[:].rearrange("p (e f) -> p e f", e=8)
                )
                tok_sizes = [128, 128, 128, 128, 32]
                for i in range(5):
                    mt = tok_sizes[i]
                    for hf in range(2):
                        pa = ppv.tile([128, 512], f32, tag="pv")
                        for e in range(8):
                            nc.tensor.matmul(
                                pa[0:mt, :],
                                xT[:, e, i * 128:i * 128 + mt],
                                wvt[:, e, hf * 512:(hf + 1) * 512],
                                start=(e == 0), stop=(e == 7 and not with_bias),
                            )
                        if with_bias:
                            nc.tensor.matmul(
                                pa[0:mt, :], oner[0:1, 0:mt].bitcast(f32r),
                                bv_sb[0:1, hf * 512:(hf + 1) * 512],
                                start=False, stop=True,
                            )
                        nc.scalar.copy(
                            vA[0:mt, i, hf * 512:(hf + 1) * 512], pa[0:mt, :]
                        )

            # per-block V slot tiles: rows 0:96 data (DRAM round-trip for the
            # partition-crossing moves), rows 96:128 buffer
            scrv = scr_pool.tile([128, 5, 1024], bf16)
            nc.sync.dma_start(scrv[:, 0:4, :], vA[:, 0:4, :])
            nc.sync.dma_start(scrv[0:32, 4, :], vA[0:32, 4, :])
            for b in range(NBLK):
                s0 = b * BLK  # global slot of block row 0 (token b*64-32)
                i0, p0 = s0 // 128, s0 % 128
                n1 = min(96, 128 - p0)
                nc.sync.dma_start(
                    vblk[0:n1, b, :], scrv[p0:p0 + n1, i0, :]
                )
                if n1 < 96:
                    nc.sync.dma_start(
                        vblk[n1:96, b, :], scrv[0:96 - n1, i0 + 1, :]
                    )
                nc.sync.dma_start(vblk[96:128, b, :], d_vbuf[:])

            # ---- attention: 16 tiles of (block b, head-octet o)
            with (
                tc.tile_pool(name="plg", bufs=2, space="PSUM") as plg,
                tc.tile_pool(name="psum_s", bufs=2, space="PSUM") as psum_s,
                tc.tile_pool(name="prb", bufs=2, space="PSUM") as prb,
            ):
                # tiles grouped by (block-pair bp, quadrant q): the 8 QK
                # matmuls of a tile (4 heads x 2 blocks) all use PE row-tile
                # q, so they serialize within one quadrant and never have two
                # row-tiles writing the same PSUM bank concurrently (HW
                # restriction). One-tile software pipeline for the
                # broadcast+normalize tail.
                pending = None
                for g in range(2 * NBLK + 1):
                    if g <= 2 * NBLK - 1:
                        bp, q = g // 4, g % 4
                        ro = q * 32
                        lg = plg.tile([128, 512], f32)
                        # mask first (start), QK accumulates on top
                        nc.tensor.matmul(
                            lg[:], ident[:], mask_sb[:, bp, :],
                            start=True, stop=False, skip_group_check=True,
                        )
                        for j in range(4):
                            for k in range(2):
                                b = 2 * bp + k
                                nc.tensor.matmul(
                                    lg[:, (j * 2 + k) * 64:(j * 2 + k + 1) * 64],
                                    kTb[ro:ro + 32, b, j, :],
                                    qT[ro:ro + 32, j, b * BLK:(b + 1) * BLK],
                                    start=False, stop=True,
                                    tile_position=(ro, 0),
                                    skip_group_check=True,
                                )
                        ex = exn[:, g, :]
                        nc.scalar.activation(
                            ex, lg[:], mybir.ActivationFunctionType.Exp
                        )
                        sums = psum_s.tile([1, 512], f32)
                        nc.tensor.matmul(
                            sums[:], onec[:, :], ex, start=True, stop=True,
                        )
                        rr = rr_pool.tile([1, 512], bf16, tag="rr16")
                        nc.vector.reciprocal(rr[:], sums[:])
                    else:
                        ex, rr = None, None
                    if pending is not None:
                        pex, prr = pending
                        rb = prb.tile([128, 512], f32)
                        nc.tensor.matmul(
                            rb[:], oner16[:], prr[:], start=True, stop=True,
                        )
                        nc.vector.tensor_tensor(
                            pex, pex, rb[:], mybir.AluOpType.mult
                        )
                    pending = (ex, rr) if ex is not None else None

                # ---- PV: one bank per head pair, evict into oT
                with tc.tile_pool(name="pov", bufs=2, space="PSUM") as pov:
                    for c in range(8):
                        ov = pov.tile([128, 512], f32)
                        for hh in range(2):
                            h = 2 * c + hh
                            for b in range(NBLK):
                                g = (b // 2) * 4 + (h % 4)
                                col = ((h // 4) * 2 + (b % 2)) * 64
                                nc.tensor.matmul(
                                    ov[hh * 64:(hh + 1) * 64,
                                       b * BLK:(b + 1) * BLK],
                                    vblk[:, b, h * 64:(h + 1) * 64],
                                    exn[:, g, col:col + 64],
                                    start=True, stop=True,
                                )
                        if c % 2 == 0:
                            nc.scalar.copy(oT[:, c, :], ov[:])
                        else:
                            nc.vector.tensor_copy(oT[:, c, :], ov[:])

            # ---- output projection: y[tok, :] = oT.T @ wp (+ bp)
            with tc.tile_pool(name="pp3", bufs=2, space="PSUM") as pp3:
                wpt = wvp_pool.tile([128, 8, 1024], bf16, tag="wp")
                nc.sync.dma_start(
                    wpt[:], d_wp[:].rearrange("p (c f) -> p c f", c=8)
                )
                for f in range(2):
                    for m in range(4):
                        pa = pp3.tile([128, 512], f32)
                        for c in range(8):
                            nc.tensor.matmul(
                                pa[:], oT[:, c, m * 128:(m + 1) * 128],
                                wpt[:, c, f * 512:(f + 1) * 512],
                                start=(c == 0),
                                stop=(c == 7 and not with_bias),
                            )
                        if with_bias:
                            nc.tensor.matmul(
                                pa[:], oner[0:1, :].bitcast(f32r),
                                bp_sb[0:1, f * 512:(f + 1) * 512],
                                start=False, stop=True,
                            )
                        ot = out_pool.tile([128, 512], f32)
                        if (f * 4 + m) % 2 == 0:
                            nc.scalar.copy(ot[:], pa[:])
                        else:
                            nc.vector.tensor_copy(ot[:], pa[:])
                        nc.sync.dma_start(
                            d_y[m * 128:(m + 1) * 128, f * 512:(f + 1) * 512],
                            ot[:],
                        )
    return nc


def _get_runner(with_bias=True):
    key = ("runner", with_bias)
    if key in _CACHE:
        return _CACHE[key]
    import jax
    import concourse.mybir as mybir
    from concourse import bass2jax
    from jax.sharding import Mesh, PartitionSpec
    from jax.experimental.shard_map import shard_map

    nc = _build_nc(with_bias)
    bass2jax.install_neuronx_cc_hook()
    partition_name = nc.partition_id_tensor.name if nc.partition_id_tensor else None
    in_names, out_names, out_avals, out_shapes = [], [], [], []
    for alloc in nc.m.functions[0].allocations:
        if not isinstance(alloc, mybir.MemoryLocationSet):
            continue
        name = alloc.memorylocations[0].name
        if alloc.kind == "ExternalInput":
            if name != partition_name:
                in_names.append(name)
        elif alloc.kind == "ExternalOutput":
            shape = tuple(alloc.tensor_shape)
            dtype = mybir.dt.np(alloc.dtype)
            out_names.append(name)
            out_avals.append(jax.core.ShapedArray(shape, dtype))
            out_shapes.append((shape, dtype))
    n_params = len(in_names)
    n_outs = len(out_avals)
    all_in_names = in_names + out_names + ([partition_name] if partition_name else [])
    donate = tuple(range(n_params, n_params + n_outs))

    def _body(*args):
        operands = list(args)
        if partition_name is not None:
            operands.append(bass2jax.partition_id_tensor())
        outs = bass2jax._bass_exec_p.bind(
            *operands,
            out_avals=tuple(out_avals),
            in_names=tuple(all_in_names),
            out_names=tuple(out_names),
            lowering_input_output_aliases=(),
            sim_require_finite=True,
            sim_require_nnan=True,
            nc=nc,
        )
        return tuple(outs)

    devices = jax.devices()[:N_CORES]
    mesh = Mesh(np.asarray(devices), ("core",))
    sharded = jax.jit(
        shard_map(
            _body, mesh=mesh,
            in_specs=(PartitionSpec("core"),) * (n_params + n_outs),
            out_specs=(PartitionSpec("core"),) * n_outs,
            check_rep=False,
        ),
        donate_argnums=donate,
        keep_unused=True,
    )

    def run(in_maps):
        per_core = [[np.asarray(m[name]) for name in in_names] for m in in_maps]
        concat_in = [
            np.concatenate([per_core[c][i] for c in range(N_CORES)], axis=0)
            for i in range(n_params)
        ]
        concat_zeros = [
            np.zeros((N_CORES * s[0], *s[1:]), d) for (s, d) in out_shapes
        ]
        out_arrs = sharded(*concat_in, *concat_zeros)
        return [
            {
                name: np.asarray(out_arrs[i]).reshape(N_CORES, *out_shapes[i][0])[c]
                for i, name in enumerate(out_names)
            }
            for c in range(N_CORES)
        ]

    _CACHE[key] = run
    return run


# ------------------------------------------------------------------- host
def _bf16(a):
    import ml_dtypes

    return np.asarray(a, np.float32).astype(ml_dtypes.bfloat16)


def _prep_inputs(x, Wkv, bkv, Wq, bq, Wp, bp, buffer, sample_lengths):
    x = np.asarray(x, np.float32)
    Wkv = np.asarray(Wkv, np.float32)
    bkv = np.asarray(bkv, np.float32)
    Wq = np.asarray(Wq, np.float32)
    bq = np.asarray(bq, np.float32)
    Wp = np.asarray(Wp, np.float32)
    bp = np.asarray(bp, np.float32)
    buffer = np.asarray(buffer, np.float32)
    lengths = np.asarray(sample_lengths).astype(np.int64)

    scale = 1.0 / math.sqrt(DK)
    starts = np.concatenate([[0], np.cumsum(lengths)[:-1]]).astype(np.int64)
    t = np.arange(T)
    seg = np.searchsorted(starts, t, side="right") - 1
    j = t - starts[seg]

    # weights in device layout
    wkT = np.ascontiguousarray(Wkv[:KD, :].T)              # [E, KD]
    wk = _bf16(wkT.reshape(8, 128, 4, 128).transpose(1, 2, 0, 3).reshape(128, -1))
    wqT = np.ascontiguousarray(Wq.T * scale)
    wq = _bf16(wqT.reshape(8, 128, 4, 128).transpose(1, 2, 0, 3).reshape(128, -1))
    wvT = np.ascontiguousarray(Wkv[KD:, :].T)              # [E, E]
    wv = _bf16(wvT.reshape(8, 128, 1024).transpose(1, 0, 2).reshape(128, -1))
    wpT = np.ascontiguousarray(Wp.T)
    wp = _bf16(wpT.reshape(8, 128, 1024).transpose(1, 0, 2).reshape(128, -1))

    bk2 = np.ascontiguousarray(bkv[:KD].reshape(4, 128).T)
    bq2 = np.ascontiguousarray((bq * scale).reshape(4, 128).T)
    bv_row = np.ascontiguousarray(bkv[KD:][None, :])
    bp_row = np.ascontiguousarray(bp[None, :])

    # buffer rows 0..30 + zero pad row 31
    buf32 = np.zeros((32, E + KD), np.float32)
    buf32[:W - 1] = buffer
    kbuf = _bf16(buf32[:, :KD].T.reshape(4, 128, 32).transpose(1, 0, 2).reshape(128, -1))
    vbuf = _bf16(buf32[:, KD:])

    ident = _bf16(np.eye(128, dtype=np.float32))
    onec = _bf16(np.ones((128, 1), np.float32))
    oner = np.ones((1, 128), np.float32)
    oner16 = _bf16(oner)

    # x feature-major with 32-token front halo
    xTp = np.zeros((E, HALO + T), np.float32)
    xTp[:, HALO:] = x.T

    in_maps = []
    for c in range(N_CORES):
        t0 = c * SHARD
        xc = xTp[:, t0:t0 + NTOK]
        xT_c = _bf16(xc.reshape(8, 128, NTOK).transpose(1, 0, 2).reshape(128, -1))

        # mask [NBLK, 128 slots, 64 tokens] -> replicate over 8 heads
        mask = np.full((NBLK, 128, BLK), NEG, np.float32)
        for bblk in range(NBLK):
            i = np.arange(BLK)
            tt = t0 + bblk * BLK + i                        # global tokens
            st = starts[seg[tt]]
            jj = j[tt]
            s = np.arange(96)[:, None]
            gs = t0 + bblk * BLK - HALO + s                 # global slot token
            valid = (
                (gs >= tt[None, :] - (W - 1)) & (gs <= tt[None, :])
                & (gs >= st[None, :]) & (gs >= 0)
            )
            mask[bblk, 0:96, :] = np.where(valid, 0.0, NEG)
            bi = np.arange(32)[:, None]
            validb = (bi >= jj[None, :]) & (bi <= W - 2)
            mask[bblk, 96:128, :] = np.where(validb, 0.0, NEG)
        # [128, 4 block-pairs, (4 heads x 2 blocks) * 64]: col (j*2+k)*64
        # holds block 2*bp+k, independent of head index j
        mpair = mask.reshape(NBLK // 2, 2, 128, BLK)     # [bp, k, p, i]
        mrep = np.tile(mpair.transpose(2, 0, 1, 3)[:, :, None, :, :],
                       (1, 1, 4, 1, 1))                  # [p, bp, j, k, i]
        mask_dev = _bf16(mrep.reshape(128, -1))

        in_maps.append({
            "xT": xT_c, "wk": wk, "wq": wq, "wv": wv, "wp": wp,
            "mask": mask_dev, "kbuf": kbuf, "vbuf": vbuf,
            "ident": ident, "onec": onec, "oner": oner, "oner16": oner16,
            "bk": bk2, "bq": bq2, "bv": bv_row, "bp": bp_row,
        })
    return in_maps, seg, j


def kernel(x, Wkv, bkv, Wq, bq, Wp, bp, buffer, sample_lengths):
    in_maps, seg, j = _prep_inputs(
        x, Wkv, bkv, Wq, bq, Wp, bp, buffer, sample_lengths
    )
    with_bias = bool(
        np.any(np.asarray(bkv)) or np.any(np.asarray(bq)) or np.any(np.asarray(bp))
    )
    run = _get_runner(with_bias)
    results = run(in_maps)
    out_full = np.concatenate([results[c]["yout"] for c in range(N_CORES)], axis=0)
    y = np.zeros((B, MAXL, E), np.float32)
    ok = j < MAXL
    y[seg[ok], j[ok]] = out_full[ok]
    return y
